# revision 1
# baseline (speedup 1.0000x reference)
# Trainium2 Bass kernel for nn_Decoder_19353122635844 (dense transformer layer).
#
# Strategy: data-parallel over batch B=128 -> 16 sequences per NeuronCore (8 cores).
# All activations are kept feature-major [D, tokens] so every matmul consumes them
# directly (zero on-chip transposes):
#   - q/k projections:   per-head out[73, tok] = Wq-head-cols (lhsT) x hT (rhs)
#   - v projection:      out[tok, feat] = hT-col-slices (lhsT) x Wv_packed (rhs);
#       an all-ones column per head (written by a strided memset) makes the
#       softmax denominator fall out of the o-matmul for free.
#   - scoresT[s,q]       = k-head (lhsT, K=73) x q-head (rhs); causal mask via
#       affine_select; exp on ACT with the 1/sqrt(D) scale folded in.
#   - o[feat, q]         = v_aug (lhsT) x exp(scoresT) (rhs); row 73 = denom.
#   - LayerNorm stats via ones-vector matmuls; normalization uses gpsimd
#       partition_broadcast of the per-token sum/rstd rows.
# fp32r (tf32-class, full PE rate at N>=256) for projections/FFN, bf16 for
# attention internals, fp32 for residuals/LN arithmetic. x2 (output of the
# second attention block) is staged through DRAM between the attention phase
# and the FFN phase to stay within SBUF.
import numpy as np
import ml_dtypes

B, T, D, H, HS = 128, 200, 584, 8, 73
FF = 4 * D              # 2336
EPS = 1e-5
SCALE = D ** -0.5
NCORES = 8
SEQ_PER_CORE = B // NCORES   # 16
S_GRP = 2                    # sequences per group
GROUPS = SEQ_PER_CORE // S_GRP
W = S_GRP * T                # 400 tokens per group
NTOK = SEQ_PER_CORE * T      # 3200
DP = 640                     # D padded to 5*128
KC = DP // 128               # 5 contraction chunks over D
FKC = (FF + 127) // 128      # 19 contraction chunks over FF
VA = H * (HS + 1)            # 592: per-head v columns + ones column
C0 = float(D) * float(D) * EPS  # bias inside sqrt for rstd rows

_CACHE = {}


def _build_nc(ln_affine=False):
    import concourse.bacc as bacc
    import concourse.tile as tile
    import concourse.mybir as mybir
    from contextlib import ExitStack

    dt = mybir.dt
    AF = mybir.ActivationFunctionType
    OP = mybir.AluOpType

    LN_AFFINE = ln_affine
    nc = bacc.Bacc(None, target_bir_lowering=False, debug=False)

    f32, f32r, bf16 = dt.float32, dt.float32r, dt.bfloat16

    xt = nc.declare_dram_parameter("xt", [KC, 128, NTOK], f32, isOutput=False)
    yt = nc.declare_dram_parameter("yt", [KC, 128, NTOK], f32, isOutput=True)
    wq_sa = nc.declare_dram_parameter("wq_sa", [KC, 128, D], f32, isOutput=False)
    wk_sa = nc.declare_dram_parameter("wk_sa", [KC, 128, D], f32, isOutput=False)
    wv_sa = nc.declare_dram_parameter("wv_sa", [KC, 128, VA], f32, isOutput=False)
    wo_sa = nc.declare_dram_parameter("wo_sa", [H, 128, D], bf16, isOutput=False)
    bo_sa = nc.declare_dram_parameter("bo_sa", [KC, 128, 1], f32, isOutput=False)
    wq_ca = nc.declare_dram_parameter("wq_ca", [KC, 128, D], f32, isOutput=False)
    wk_ca = nc.declare_dram_parameter("wk_ca", [KC, 128, D], f32, isOutput=False)
    wv_ca = nc.declare_dram_parameter("wv_ca", [KC, 128, VA], f32, isOutput=False)
    wo_ca = nc.declare_dram_parameter("wo_ca", [H, 128, D], bf16, isOutput=False)
    bo_ca = nc.declare_dram_parameter("bo_ca", [KC, 128, 1], f32, isOutput=False)
    w1 = nc.declare_dram_parameter("w1", [KC, 128, FF], f32, isOutput=False)
    b1 = nc.declare_dram_parameter("b1", [FKC, 128, 1], f32, isOutput=False)
    w2 = nc.declare_dram_parameter("w2", [FKC, 128, D], bf16, isOutput=False)
    b2 = nc.declare_dram_parameter("b2", [KC, 128, 1], f32, isOutput=False)
    g1p = nc.declare_dram_parameter("g1p", [KC, 128, 1], f32, isOutput=False)
    be1 = nc.declare_dram_parameter("be1", [KC, 128, 1], f32, isOutput=False)
    g2p = nc.declare_dram_parameter("g2p", [KC, 128, 1], f32, isOutput=False)
    onesd = nc.declare_dram_parameter("onesd", [128, 1], f32, isOutput=False)
    be2 = nc.declare_dram_parameter("be2", [KC, 128, 1], f32, isOutput=False)

    # DRAM staging for x2 between the attention phase and the FFN phase
    x2d = nc.dram_tensor("x2d", [KC, 128, NTOK], f32)

    with tile.TileContext(nc) as tc, ExitStack() as ctx:
        const = ctx.enter_context(tc.tile_pool(name="const", bufs=1))
        pp_proj = ctx.enter_context(tc.tile_pool(name="pp_proj", bufs=2, space="PSUM"))
        pp_att = ctx.enter_context(tc.tile_pool(name="pp_att", bufs=4, space="PSUM"))
        pp_st = ctx.enter_context(tc.tile_pool(name="pp_st", bufs=1, space="PSUM"))

        ones = const.tile([128, 1], f32r)
        nc.sync.dma_start(out=ones[:], in_=onesd[:, :].bitcast(f32r))
        c0t = const.tile([1, 1], f32)
        nc.vector.memset(c0t[:], C0)

        def load_vec(dram, n):
            t = const.tile([128, n, 1], f32, tag=dram.name + "_sb")
            for k in range(n):
                nc.sync.dma_start(out=t[:, k, :], in_=dram[k, :, :])
            return t

        g1s = load_vec(g1p, KC)
        be1s = load_vec(be1, KC)
        g2s = load_vec(g2p, KC)
        be2s = load_vec(be2, KC)
        bosas = load_vec(bo_sa, KC)
        bocas = load_vec(bo_ca, KC)
        b2s = load_vec(b2, KC)
        b1s = load_vec(b1, FKC)

        def layer_norm(src_ap_fn, gs, bes, dst, sq, act, affine=LN_AFFINE):
            """src_ap_fn(k) -> [128, W] f32r AP (rows past D are zero).
            dst: [128, KC, W] f32r tile. sq: scratch tile [128, KC, W] f32r."""
            for k in range(KC):
                nc.scalar.activation(out=sq[:, k, :], in_=src_ap_fn(k), func=AF.Square)
            u_ps = pp_st.tile([1, W], f32, tag="st_u")
            w_ps = pp_st.tile([1, W], f32, tag="st_w")
            for k in range(KC):
                nc.tensor.matmul(u_ps[:], ones[:], src_ap_fn(k),
                                 start=(k == 0), stop=(k == KC - 1))
            for k in range(KC):
                nc.tensor.matmul(w_ps[:], ones[:], sq[:, k, :],
                                 start=(k == 0), stop=(k == KC - 1))
            urow = act.tile([1, W], f32, tag="urow", bufs=2)
            nc.scalar.activation(out=urow[:], in_=u_ps[:], func=AF.Copy)
            vrow = act.tile([1, W], f32, tag="vrow", bufs=2)
            nc.vector.tensor_mul(vrow[:], urow[:], urow[:])
            trow = act.tile([1, W], f32, tag="trow", bufs=2)
            nc.vector.scalar_tensor_tensor(out=trow[:], in0=w_ps[:], scalar=float(D),
                                           in1=vrow[:], op0=OP.mult, op1=OP.subtract)
            lnrow = act.tile([1, W], f32, tag="lnrow", bufs=2)
            nc.scalar.activation(out=lnrow[:], in_=trow[:], func=AF.Ln, bias=c0t[:])
            rrow = act.tile([1, W], f32, tag="rrow", bufs=2)
            nc.scalar.activation(out=rrow[:], in_=lnrow[:], func=AF.Exp, scale=-0.5)
            ub = act.tile([128, W], f32, tag="ub", bufs=2)
            rb = act.tile([128, W], f32, tag="rb", bufs=2)
            nc.gpsimd.partition_broadcast(ub[:], urow[:])
            nc.gpsimd.partition_broadcast(rb[:], rrow[:])
            # dst = ((D*x - u) * g) * r + be ; r = 1/sqrt(D*sumsq - u^2 + D^2*eps)
            for k in range(KC):
                dsc = act.tile([128, W], f32, tag="lnd", bufs=2)
                nc.vector.scalar_tensor_tensor(out=dsc[:], in0=src_ap_fn(k),
                                               scalar=float(D), in1=ub[:],
                                               op0=OP.mult, op1=OP.subtract)
                if affine:
                    esc = act.tile([128, W], f32, tag="lne", bufs=2)
                    nc.vector.scalar_tensor_tensor(out=esc[:], in0=dsc[:],
                                                   scalar=gs[:, k, :], in1=rb[:],
                                                   op0=OP.mult, op1=OP.mult)
                    nc.scalar.activation(out=dst[:, k, :], in_=esc[:],
                                         func=AF.Identity, bias=bes[:, k, :])
                else:
                    nc.vector.scalar_tensor_tensor(out=dst[:, k, :], in0=dsc[:],
                                                   scalar=1.0, in1=rb[:],
                                                   op0=OP.mult, op1=OP.mult)

        def attn_block(ht, resid, wq_sb, wk_sb, wv_sb, wo_sb, bo_sb, out_dst, act):
            """One causal-MHA block on one group of S_GRP sequences.
            ht: [128, KC, W] f32r input (feature-major; rows past D are zero).
            resid: None or [128, KC, W] f32r tile added after the out-proj.
            out_dst(m, Mr) -> AP [Mr, W] destination for out-proj chunk m.
            """
            # --- q/k projections, per-head M-tiles so heads sit at partition 0
            qh = [act.tile([HS, W], bf16, tag=f"qh{h}", name=f"qh{h}", bufs=2) for h in range(H)]
            kh = [act.tile([HS, W], bf16, tag=f"kh{h}", name=f"kh{h}", bufs=2) for h in range(H)]
            for wsb, dst in ((wq_sb, qh), (wk_sb, kh)):
                for h in range(H):
                    p = pp_proj.tile([128, W], f32, tag="proj")
                    for k in range(KC):
                        nc.tensor.matmul(p[0:HS, :], wsb[:, k, HS * h:HS * h + HS],
                                         ht[:, k, :], start=(k == 0), stop=(k == KC - 1))
                    nc.scalar.activation(out=dst[h][:], in_=p[0:HS, :], func=AF.Copy)
            # --- v projection (token-major), ones-columns via strided memset
            vas = []
            for st in range(2 * S_GRP):
                s, half = st // 2, st % 2
                Mr = 128 if half == 0 else T - 128
                cols = 200 * s + 128 * half
                va = act.tile([128, VA], bf16, tag=f"va{st}", name=f"va{st}")
                VH = VA // 2  # 296 (>=256 keeps fp32r at full rate)
                pa = pp_proj.tile([128, 512], f32, tag="proj")
                pb = pp_proj.tile([128, 512], f32, tag="proj")
                for k in range(KC):
                    nc.tensor.matmul(pa[0:Mr, 0:VH], ht[:, k, cols:cols + Mr],
                                     wv_sb[:, k, 0:VH], start=(k == 0),
                                     stop=(k == KC - 1))
                for k in range(KC):
                    nc.tensor.matmul(pb[0:Mr, 0:VH], ht[:, k, cols:cols + Mr],
                                     wv_sb[:, k, VH:VA], start=(k == 0),
                                     stop=(k == KC - 1))
                nc.scalar.activation(out=va[0:Mr, 0:VH], in_=pa[0:Mr, 0:VH], func=AF.Copy)
                nc.scalar.activation(out=va[0:Mr, VH:VA], in_=pb[0:Mr, 0:VH],
                                     func=AF.Copy)
                nc.vector.memset(va[0:Mr, HS:VA:HS + 1], 1.0)
                vas.append(va)
            # --- attention per (sequence, head)
            och = [act.tile([HS, W], bf16, tag=f"oc{h}", name=f"oc{h}", bufs=2) for h in range(H)]
            for s in range(S_GRP):
                for h in range(H):
                    sc = pp_att.tile([128, 272], f32, tag="att")
                    nc.tensor.matmul(sc[:, 0:200], kh[h][:, 200 * s:200 * s + 128],
                                     qh[h][:, 200 * s:200 * s + 200],
                                     start=True, stop=True)
                    nc.tensor.matmul(sc[0:72, 200:272],
                                     kh[h][:, 200 * s + 128:200 * s + 200],
                                     qh[h][:, 200 * s + 128:200 * s + 200],
                                     start=True, stop=True, skip_group_check=True)
                    w0 = act.tile([128, 200], bf16, tag="w0", bufs=4)
                    w1t = act.tile([72, 72], bf16, tag="w1t", bufs=4)
                    nc.scalar.activation(out=w0[:], in_=sc[:, 0:200], func=AF.Exp,
                                         scale=SCALE)
                    nc.scalar.activation(out=w1t[:], in_=sc[0:72, 200:272], func=AF.Exp,
                                         scale=SCALE)
                    nc.gpsimd.affine_select(out=w0[:, 0:128], in_=w0[:, 0:128],
                                            pattern=[[1, 128]], compare_op=OP.is_ge,
                                            fill=0.0, base=0, channel_multiplier=-1)
                    nc.gpsimd.affine_select(out=w1t[:], in_=w1t[:],
                                            pattern=[[1, 72]], compare_op=OP.is_ge,
                                            fill=0.0, base=0, channel_multiplier=-1)
                    va0, va1 = vas[2 * s], vas[2 * s + 1]
                    vc = (HS + 1) * h
                    o = pp_att.tile([HS, 200], f32, tag="att")
                    nc.tensor.matmul(o[:, 128:200], va0[:, vc:vc + HS],
                                     w0[:, 128:200], start=True, stop=False)
                    nc.tensor.matmul(o[:, 128:200], va1[0:72, vc:vc + HS],
                                     w1t[:], start=False, stop=False)
                    nc.tensor.matmul(o[:, 0:128], va0[:, vc:vc + HS],
                                     w0[:, 0:128], start=False, stop=True)
                    # denominator row via the ones-column (M=1 matmuls)
                    dn = pp_att.tile([1, 200], f32, tag="att")
                    nc.tensor.matmul(dn[:, 128:200], va0[:, vc + HS:vc + HS + 1],
                                     w0[:, 128:200], start=True, stop=False)
                    nc.tensor.matmul(dn[:, 128:200], va1[0:72, vc + HS:vc + HS + 1],
                                     w1t[:], start=False, stop=False)
                    nc.tensor.matmul(dn[:, 0:128], va0[:, vc + HS:vc + HS + 1],
                                     w0[:, 0:128], start=False, stop=True)
                    rr = act.tile([1, 200], f32, tag="arec", bufs=4)
                    nc.vector.reciprocal(out=rr[:], in_=dn[:])
                    arb = act.tile([HS, 200], f32, tag="arb", bufs=4)
                    nc.gpsimd.partition_broadcast(arb[:], rr[:])
                    nc.vector.tensor_mul(och[h][:, 200 * s:200 * s + 200],
                                         o[0:HS, :], arb[:])
            # --- output projection (+bias, +residual), K=73 per head chunk
            for m in range(KC):
                Mr = 128 if m < KC - 1 else D - 128 * (KC - 1)
                p = pp_proj.tile([128, W], f32, tag="proj")
                for h in range(H):
                    nc.tensor.matmul(p[0:Mr, :], wo_sb[0:HS, h, 128 * m:128 * m + Mr],
                                     och[h][:], start=(h == 0), stop=(h == H - 1))
                dst = out_dst(m, Mr)
                if resid is not None:
                    nc.vector.scalar_tensor_tensor(out=dst, in0=p[0:Mr, :],
                                                   scalar=bo_sb[0:Mr, m, :],
                                                   in1=resid[0:Mr, m, :],
                                                   op0=OP.add, op1=OP.add)
                else:
                    nc.vector.tensor_scalar_add(dst, p[0:Mr, :], bo_sb[0:Mr, m, :])

        # ---------------- Phase A: both attention blocks, all groups ----------------
        with tc.tile_pool(name="wattn", bufs=1) as wp, \
             tc.tile_pool(name="actA", bufs=1) as act:
            def load_w(dram, n, cols, dtype, cast):
                t = wp.tile([128, n, cols], dtype, tag=dram.name + "_sb")
                for k in range(n):
                    src = dram[k, :, :]
                    if cast:
                        src = src.bitcast(f32r)
                    nc.sync.dma_start(out=t[:, k, :], in_=src)
                return t

            wqs = load_w(wq_sa, KC, D, f32r, True)
            wks = load_w(wk_sa, KC, D, f32r, True)
            wvs = load_w(wv_sa, KC, VA, f32r, True)
            wos = load_w(wo_sa, H, D, bf16, False)
            wqc = load_w(wq_ca, KC, D, f32r, True)
            wkc = load_w(wk_ca, KC, D, f32r, True)
            wvc = load_w(wv_ca, KC, VA, f32r, True)
            woc = load_w(wo_ca, H, D, bf16, False)

            for g in range(GROUPS):
                c0 = g * W
                xg = act.tile([128, KC, W], f32r, tag="xg", bufs=2)
                for k in range(KC):
                    nc.sync.dma_start(out=xg[:, k, :],
                                      in_=xt[k, :, c0:c0 + W].bitcast(f32r))
                sq = act.tile([128, KC, W], f32r, tag="asq")
                h1 = act.tile([128, KC, W], f32r, tag="h1")
                layer_norm(lambda k: xg[:, k, :], g1s, be1s, h1, sq, act)

                x1g = act.tile([128, KC, W], f32r, tag="asq")  # reuses sq slot
                # zero the pad rows of the last chunk once per group (the
                # out-proj epilogue below only writes rows 0:72 of chunk 4)
                nc.vector.memset(x1g[64:128, KC - 1, :].bitcast(f32), 0.0)
                attn_block(h1, xg, wqs, wks, wvs, wos, bosas,
                           lambda m, Mr: x1g[0:Mr, m, :], act)

                x2g = act.tile([128, KC, W], f32r, tag="h1")  # reuses h1 slot
                nc.vector.memset(x2g[64:128, KC - 1, :].bitcast(f32), 0.0)
                attn_block(x1g, None, wqc, wkc, wvc, woc, bocas,
                           lambda m, Mr: x2g[0:Mr, m, :], act)
                for k in range(KC):
                    nc.gpsimd.dma_start(out=x2d[k, :, c0:c0 + W].bitcast(f32r),
                                        in_=x2g[:, k, :])

        # ---------------- Phase B: FFN, all groups ----------------
        with tc.tile_pool(name="wffn", bufs=1) as wf, \
             tc.tile_pool(name="actB", bufs=1) as act:
            w1s = wf.tile([128, KC, FF], f32r, tag="w1_sb")
            for k in range(KC):
                nc.sync.dma_start(out=w1s[:, k, :], in_=w1[k, :, :].bitcast(f32r))
            w2s = wf.tile([128, FKC, D], bf16, tag="w2_sb")
            for k in range(FKC):
                nc.sync.dma_start(out=w2s[:, k, :], in_=w2[k, :, :])

            for g in range(GROUPS):
                c0 = g * W
                x2g = act.tile([128, KC, W], f32r, tag="x2g")
                for k in range(KC):
                    nc.gpsimd.dma_start(out=x2g[:, k, :],
                                        in_=x2d[k, :, c0:c0 + W].bitcast(f32r))
                sq = act.tile([128, KC, W], f32r, tag="bsq")
                h2 = act.tile([128, KC, W], f32r, tag="h2")
                layer_norm(lambda k: x2g[:, k, :], g2s, be2s, h2, sq, act)
                relu = act.tile([128, FKC, W], bf16, tag="relu")
                FR = FF - 128 * (FKC - 1)  # 32 valid rows in the last chunk
                nc.vector.memset(relu[32:64, FKC - 1, :], 0.0)
                nc.vector.memset(relu[64:128, FKC - 1, :], 0.0)
                for mf in range(FKC):
                    Mr = 128 if mf < FKC - 1 else FR
                    p = pp_proj.tile([128, W], f32, tag="proj")
                    for k in range(KC):
                        nc.tensor.matmul(p[0:Mr, :], w1s[:, k, 128 * mf:128 * mf + Mr],
                                         h2[:, k, :], start=(k == 0), stop=(k == KC - 1))
                    nc.scalar.activation(out=relu[0:Mr, mf, :], in_=p[0:Mr, :],
                                         func=AF.Relu, bias=b1s[0:Mr, mf, :])
                outg = act.tile([128, KC, W], f32, tag="outg")
                for m in range(KC):
                    Mr = 128 if m < KC - 1 else D - 128 * (KC - 1)
                    p = pp_proj.tile([128, W], f32, tag="proj")
                    for k in range(FKC):
                        nc.tensor.matmul(p[0:Mr, :], w2s[:, k, 128 * m:128 * m + Mr],
                                         relu[:, k, :], start=(k == 0), stop=(k == FKC - 1))
                    nc.vector.scalar_tensor_tensor(out=outg[0:Mr, m, :], in0=p[0:Mr, :],
                                                   scalar=b2s[0:Mr, m, :],
                                                   in1=x2g[0:Mr, m, :],
                                                   op0=OP.add, op1=OP.add)
                for k in range(KC):
                    Mr = 128 if k < KC - 1 else D - 128 * (KC - 1)
                    nc.sync.dma_start(out=yt[k, 0:Mr, c0:c0 + W], in_=outg[0:Mr, k, :])

    nc.compile()
    return nc


def _pad_rows(a, rows):
    out = np.zeros((rows,) + a.shape[1:], dtype=np.float32)
    out[: a.shape[0]] = a
    return out


def pack_weights(inputs):
    f = lambda x: np.asarray(x, dtype=np.float32)
    bf = ml_dtypes.bfloat16

    def packqk(w):  # [H, D, HS] -> [KC, 128, D] (rows d padded, cols head-major)
        a = f(w).transpose(1, 0, 2).reshape(D, D)
        return _pad_rows(a, DP).reshape(KC, 128, D)

    def packv(w):  # [H, D, HS] -> [KC, 128, VA]; ones-columns stay zero here
        a = np.zeros((DP, VA), dtype=np.float32)
        w = f(w)
        for h in range(H):
            a[0:D, (HS + 1) * h:(HS + 1) * h + HS] = w[h]
        return a.reshape(KC, 128, VA)

    def packo(w):  # [D, D] -> [H, 128, D] (chunk h rows 0:73 = Wo[73h:73h+73])
        a = np.zeros((H, 128, D), dtype=np.float32)
        w = f(w)
        for h in range(H):
            a[h, 0:HS, :] = w[HS * h:HS * h + HS, :]
        return a

    def vec(v, n):  # [len] -> [n, 128, 1]
        return _pad_rows(f(v).reshape(-1, 1), n * 128).reshape(n, 128, 1)

    wm = {
        "wq_sa": packqk(inputs["Wq_sa"]),
        "wk_sa": packqk(inputs["Wk_sa"]),
        "wv_sa": packv(inputs["Wv_sa"]),
        "wo_sa": packo(inputs["Wo_sa"]).astype(bf),
        "bo_sa": vec(inputs["bo_sa"], KC),
        "wq_ca": packqk(inputs["Wq_ca"]),
        "wk_ca": packqk(inputs["Wk_ca"]),
        "wv_ca": packv(inputs["Wv_ca"]),
        "wo_ca": packo(inputs["Wo_ca"]).astype(bf),
        "bo_ca": vec(inputs["bo_ca"], KC),
        "w1": _pad_rows(f(inputs["W1"]), DP).reshape(KC, 128, FF),
        "b1": vec(inputs["b1"], FKC),
        "w2": _pad_rows(f(inputs["W2"]), FKC * 128).reshape(FKC, 128, D).astype(bf),
        "b2": vec(inputs["b2"], KC),
        "g1p": vec(inputs["g1"], KC),
        "be1": vec(inputs["be1"], KC),
        "g2p": vec(inputs["g2"], KC),
        "onesd": np.ones((128, 1), dtype=np.float32),
        "be2": vec(inputs["be2"], KC),
    }
    return wm


def pack_x_core(x, c):
    xc = np.asarray(x[SEQ_PER_CORE * c:SEQ_PER_CORE * (c + 1)],
                    dtype=np.float32).reshape(NTOK, D)
    return np.ascontiguousarray(
        _pad_rows(np.ascontiguousarray(xc.T), DP).reshape(KC, 128, NTOK))


def unpack_y(yts):
    out = np.empty((B, T, D), dtype=np.float32)
    for c, y in enumerate(yts):
        o = np.asarray(y).reshape(DP, NTOK)[0:D].T  # [NTOK, D]
        out[SEQ_PER_CORE * c:SEQ_PER_CORE * (c + 1)] = o.reshape(SEQ_PER_CORE, T, D)
    return out


def get_nc(ln_affine=False):
    key = ("nc", ln_affine)
    if key not in _CACHE:
        _CACHE[key] = _build_nc(ln_affine)
    return _CACHE[key]


def kernel(**inputs):
    from concourse.bass_utils import run_bass_kernel_spmd

    affine = not (np.all(np.asarray(inputs["g1"]) == 1.0)
                  and np.all(np.asarray(inputs["g2"]) == 1.0)
                  and np.all(np.asarray(inputs["be1"]) == 0.0)
                  and np.all(np.asarray(inputs["be2"]) == 0.0))
    nc = get_nc(ln_affine=affine)
    wm = pack_weights(inputs)
    in_maps = [dict(wm, xt=pack_x_core(inputs["x"], c)) for c in range(NCORES)]
    res = run_bass_kernel_spmd(nc, in_maps, list(range(NCORES)))
    return unpack_y([res.results[c]["yt"] for c in range(NCORES)])

